# revision 8
# baseline (speedup 1.0000x reference)
import sys

sys.path.insert(0, "/opt/trn_rl_repo")

import numpy as np
import ml_dtypes

import concourse.bass as bass
import concourse.mybir as mybir
import concourse.tile as tile
from concourse import bacc
from concourse.bass_utils import run_bass_kernel_spmd

BF16 = ml_dtypes.bfloat16
FP8 = ml_dtypes.float8_e4m3
F32 = mybir.dt.float32
BF = mybir.dt.bfloat16
F8 = mybir.dt.float8e4
ALU = mybir.AluOpType
ACTF = mybir.ActivationFunctionType
AX = mybir.AxisListType
DR = mybir.MatmulPerfMode.DoubleRow

NCORES = 8
B = 256
BL = B // NCORES          # 32 local batch
REC = 102400
RECL = REC // NCORES      # 12800 local output cols
NW = RECL // 512          # 25 output windows
NPRE = 12                 # w3 windows prefetched before the w3 loop

S1 = 64.0                 # primcaps weight scale (fp8)
S3 = 16.0                 # w3 weight scale (fp8)
SH = 1024.0               # h2 activation scale (fp8)


def mkap(t, offset, dims):
    """Manual access pattern: dims = [[stride, count], ...] (partition dim first)."""
    return bass.AP(tensor=t.tensor if isinstance(t, bass.AP) else t, offset=offset, ap=dims)


def build_program(use_b3):
    nc = bacc.Bacc(None, num_devices=NCORES)
    rg = [list(range(NCORES))]

    P = {}
    P["pat1h"] = nc.declare_dram_parameter("pat1h", [81, 4608], BF, isOutput=False)
    P["w1c"] = nc.declare_dram_parameter("w1c", [81, 256], BF, isOutput=False)
    P["b1c2"] = nc.declare_dram_parameter("b1c2", [128, 2], F32, isOutput=False)
    P["bp22"] = nc.declare_dram_parameter("bp22", [128, 2], F32, isOutput=False)
    P["wp2q"] = nc.declare_dram_parameter("wp2q", [20736, 256], F8, isOutput=False)
    P["w2s"] = nc.declare_dram_parameter("w2s", [128, 20480], F8, isOutput=False)
    P["w2c"] = nc.declare_dram_parameter("w2c", [128, 5120], BF, isOutput=False)
    P["m4"] = nc.declare_dram_parameter("m4", [128, 4], BF, isOutput=False)
    P["rep4"] = nc.declare_dram_parameter("rep4", [BL, 128], F32, isOutput=False)
    P["ones32"] = nc.declare_dram_parameter("ones32", [BL, 1], F32, isOutput=False)
    P["ones128"] = nc.declare_dram_parameter("ones128", [128, 1], BF, isOutput=False)
    P["onesrow"] = nc.declare_dram_parameter("onesrow", [1, 128], BF, isOutput=False)
    P["id32"] = nc.declare_dram_parameter("id32", [32, 32], F32, isOutput=False)
    P["w1t"] = nc.declare_dram_parameter("w1t", [160, 512], BF, isOutput=False)
    P["b1dh"] = nc.declare_dram_parameter("b1dh", [128, 4], F32, isOutput=False)
    P["w2t"] = nc.declare_dram_parameter("w2t", [512, 1024], BF, isOutput=False)
    P["b2dh"] = nc.declare_dram_parameter("b2dh", [128, 8], F32, isOutput=False)
    P["w3q"] = nc.declare_dram_parameter("w3q", [1024, RECL], F8, isOutput=False)
    if use_b3:
        P["b3s"] = nc.declare_dram_parameter("b3s", [1, RECL], BF, isOutput=False)
    out_ext = nc.declare_dram_parameter("out", [B, RECL], BF, isOutput=True)

    with tile.TileContext(nc) as tc:
        _body(nc, tc, P, out_ext, rg, use_b3)
    nc.compile()
    return nc


def _body(nc, tc, P, out_ext, rg, use_b3):
    es = tc.tile_pool(name="const", bufs=1)
    const = es.__enter__()
    dram_cm = tc.tile_pool(name="dram", bufs=1, space="DRAM")
    dram = dram_cm.__enter__()
    w3cm = tc.tile_pool(name="w3p", bufs=NPRE)
    w3p = w3cm.__enter__()

    # ---------- warmup collective: absorb cc-stream cold-start during front ----
    wu_sb = const.tile([1, 16], F32, tag="wu", name="wu")
    nc.gpsimd.memset(wu_sb[:], 0)
    wuin = dram.tile([1, 16], F32, tag="wuin", name="wuin")
    wuout = dram.tile([NCORES, 16], F32, tag="wuout", name="wuout")
    nc.gpsimd.dma_start(wuin[:], wu_sb[:])
    nc.gpsimd.collective_compute(
        "AllGather", ALU.bypass, replica_groups=rg,
        ins=[wuin[:].opt()], outs=[wuout[:].opt()])

    # ---------- constants (conv1-critical first on the sync ring) ----------
    w1c_sb = const.tile([81, 256], BF, tag="w1c", name="w1c")
    nc.sync.dma_start(w1c_sb[:], P["w1c"][:])
    b1c2_sb = const.tile([128, 2], F32, tag="b1c2", name="b1c2")
    nc.sync.dma_start(b1c2_sb[:], P["b1c2"][:])
    bp22_sb = const.tile([128, 2], F32, tag="bp22", name="bp22")
    nc.scalar.dma_start(bp22_sb[:], P["bp22"][:])

    # persistent mid-size tiles
    uhat_sb = const.tile([128, 20480], BF, tag="uhat", name="uhat")   # [(jm,b),(m,rr,c,o)]
    xT2_sb = const.tile([128, 1024], BF, tag="xT2", name="xT2")       # [r%128,(i,jm,b)]
    xTbd_sb = const.tile([128, 4096], BF, tag="xTbd", name="xTbd")    # block-diag [.., (m, col)]
    h1T_sb = const.tile([128, 128], BF, tag="h1T", name="h1T")
    h2T_sb = const.tile([128, 256], F8, tag="h2T", name="h2T")
    w2c_sb = const.tile([128, 5120], BF, tag="w2c", name="w2c")
    nc.scalar.dma_start(w2c_sb[:], P["w2c"][:])

    xdram = dram.tile([2, 128, 512], BF, tag="xdram", name="xdram")
    bdram = dram.tile([4, 1280], BF, tag="bdram", name="bdram")
    h2loc = dram.tile([8, 128, BL], F8, tag="h2loc", name="h2loc")
    h2all = dram.tile([NCORES, 8, 128, BL], F8, tag="h2all", name="h2all")

    # =================== conv1 + primary caps (fp8 DoubleRow) ===================
    with tc.tile_pool(name="front", bufs=1) as front, \
         tc.tile_pool(name="ps_f", bufs=2, space="PSUM") as ps_f:
        pat1 = front.tile([81, 4608], BF, tag="pat1", name="pat1")
        nc.sync.dma_start(pat1[:], P["pat1h"][:])
        # primcaps weights fully resident: no mid-loop weight stalls
        wpqall = front.tile([128, 41472], F8, tag="wpqall", name="wpqall")
        for bt in range(9):
            src = mkap(P["wp2q"], bt * 9 * 2 * 128 * 256,
                       [[256, 128], [2 * 128 * 256, 9], [128 * 256, 2], [1, 256]])
            nc.sync.dma_start(
                wpqall[:, bt * 4608:(bt + 1) * 4608].rearrange(
                    "p (qq i co) -> p qq i co", qq=9, i=2), src)
        # H8: [128, (cih 2, y 12, x 12, b 32)] fp8
        H8 = front.tile([128, 9216], F8, tag="H8", name="H8")
        for h in range(2):
            for w in range(9):
                ps = ps_f.tile([128, 512], F32, tag="c1ps", name="c1ps")
                nc.tensor.matmul(ps[:], w1c_sb[:, h * 128:(h + 1) * 128],
                                 pat1[:, w * 512:(w + 1) * 512],
                                 start=True, stop=True)
                nc.scalar.activation(H8[:, h * 4608 + w * 512:h * 4608 + (w + 1) * 512],
                                     ps[:], ACTF.Relu, bias=b1c2_sb[:, h:h + 1], scale=1.0)
        # primary caps: 81 (dy,dx) pairs, DoubleRow over ci halves
        U = [front.tile([128, 512], F32, tag=f"U{h}", name=f"U{h}") for h in range(2)]
        psU = [ps_f.tile([128, 512], F32, tag=f"Ups{h}", name=f"Ups{h}", bufs=1) for h in range(2)]
        H8v = H8[:].rearrange("p (ci y x b) -> p ci y x b", ci=2, y=12, x=12)
        wv_all = wpqall[:].rearrange("p (bt qq i co) -> p bt qq i co", bt=9, qq=9, i=2)
        usq = front.tile([128, 512], F32, tag="usq", name="usq")
        sn = front.tile([128, 64], F32, tag="sn", name="sn")
        g = front.tile([128, 64], F32, tag="g", name="g")
        gt = front.tile([128, 64], F32, tag="gt", name="gt")
        X = [front.tile([128, 512], BF, tag=f"X{h}", name=f"X{h}") for h in range(2)]

        def squash_h(h):
            # squash -> x (bf16) -> DRAM; h=0 overlaps the h=1 matmul pass
            nc.scalar.activation(U[h][:], psU[h][:], ACTF.Identity,
                                 bias=bp22_sb[:, h:h + 1], scale=1.0 / S1)
            nc.vector.tensor_tensor(usq[:], U[h][:], U[h][:], op=ALU.mult)
            uview = usq[:].rearrange("p (g i b) -> p g b i", g=2, i=8)
            nc.vector.tensor_reduce(sn[:].rearrange("p (g b) -> p g b", g=2),
                                    uview, axis=AX.X, op=ALU.add)
            nc.scalar.activation(gt[:], sn[:], ACTF.Sqrt)
            nc.vector.tensor_scalar_add(g[:], sn[:], 1.0)
            nc.vector.reciprocal(g[:], g[:])
            nc.vector.tensor_tensor(g[:], g[:], gt[:], op=ALU.mult)
            gb = g[:].rearrange("p (g b) -> p g b", g=2).unsqueeze(2).broadcast_to(
                [128, 2, 8, BL])
            nc.vector.tensor_tensor(X[h][:].rearrange("p (g i b) -> p g i b", g=2, i=8),
                                    U[h][:].rearrange("p (g i b) -> p g i b", g=2, i=8),
                                    gb, op=ALU.mult)
            nc.sync.dma_start(xdram[h], X[h][:])

        # h-outer: finish all 81 taps for output-half h, then squash it while
        # the other half's matmuls run
        for h in range(2):
            for bt in range(9):
                for qq in range(9):
                    q = bt * 9 + qq
                    dy, dx = divmod(q, 9)
                    rhs = H8v[:, :, dy:dy + 4, dx:dx + 4, :]
                    nc.tensor.matmul(psU[h][:], wv_all[:, bt, qq, :, h * 128:(h + 1) * 128],
                                     rhs, start=(q == 0), stop=(q == 80),
                                     perf_mode=DR)
            squash_h(h)
        # xTbd: block-diag [(jm-blk rows (rr,i)), (m 32, cols 128)] -- u_hat critical
        nc.vector.memset(xTbd_sb[:], 0)
        for jm in range(4):
            dst = xTbd_sb[32 * jm:32 * (jm + 1), :].rearrange(
                "p (m c) -> p m c", m=32)[:, :, 32 * jm:32 * (jm + 1)]
            src = mkap(xdram[:], jm * 32768, [[32, 32], [1024, 32], [1, 32]])
            nc.sync.dma_start(dst, src)
        # xT2[p, (i 8, jm 4, b 32)] = x[b, jm*128+p, i]
        xsrc2 = mkap(xdram[:], 0, [[256, 128], [128, 2], [32, 4], [32768, 4], [1, 32]])
        nc.sync.dma_start(
            xT2_sb[:].rearrange("p (y0 x jm b) -> p y0 x jm b", y0=2, x=4, jm=4), xsrc2)

    m4_sb = const.tile([128, 4], BF, tag="m4", name="m4")
    nc.scalar.dma_start(m4_sb[:], P["m4"][:])
    rep4_sb = const.tile([BL, 128], F32, tag="rep4", name="rep4")
    nc.scalar.dma_start(rep4_sb[:], P["rep4"][:])
    ones32_sb = const.tile([BL, 1], F32, tag="ones32", name="ones32")
    nc.scalar.dma_start(ones32_sb[:], P["ones32"][:])
    ones128_sb = const.tile([128, 1], BF, tag="ones128", name="ones128")
    nc.scalar.dma_start(ones128_sb[:], P["ones128"][:])
    onesrow_sb = const.tile([1, 128], BF, tag="onesrow", name="onesrow")
    nc.scalar.dma_start(onesrow_sb[:], P["onesrow"][:])
    id32_sb = const.tile([32, 32], F32, tag="id32", name="id32")
    nc.scalar.dma_start(id32_sb[:], P["id32"][:])
    w1ta_sb = const.tile([128, 512], BF, tag="w1ta", name="w1ta")
    nc.scalar.dma_start(w1ta_sb[:], P["w1t"][0:128, :])
    w1tb_sb = const.tile([32, 512], BF, tag="w1tb", name="w1tb")
    nc.scalar.dma_start(w1tb_sb[:], P["w1t"][128:160, :])
    b1dh_sb = const.tile([128, 4], F32, tag="b1dh", name="b1dh")
    nc.scalar.dma_start(b1dh_sb[:], P["b1dh"][:])
    b2dh_sb = const.tile([128, 8], F32, tag="b2dh", name="b2dh")
    nc.scalar.dma_start(b2dh_sb[:], P["b2dh"][:])
    w2th_sb = const.tile([128, 4096], BF, tag="w2th", name="w2th")
    nc.scalar.dma_start(w2th_sb[:].rearrange("p (kc n) -> p kc n", kc=4),
                        mkap(P["w2t"], 0, [[1024, 128], [128 * 1024, 4], [1, 1024]]))

    def w3_load(w, wt3):
        wsrc = mkap(P["w3q"], w * 512,
                    [[RECL, 128], [256 * RECL, 4], [128 * RECL, 2], [1, 512]])
        nc.sync.dma_start(wt3[:].rearrange("p (q i n) -> p q i n", q=4, i=2), wsrc)

    # ---------- w3 prefetch: NPRE window loads on the sync ring ----------
    w3tiles = []
    for w in range(NPRE):
        wt3 = w3p.tile([128, 4096], F8, tag="w3t", name="w3t")
        w3_load(w, wt3)
        w3tiles.append(wt3)

    # =================== routing pools (opened early: it0 overlaps u_hat) ======
    rt_cm = tc.tile_pool(name="route", bufs=1)
    rt = rt_cm.__enter__()
    psr_cm = tc.tile_pool(name="ps_r", bufs=1, space="PSUM")
    ps_r = psr_cm.__enter__()
    if True:
        cwt = rt.tile([128, 5120], BF, tag="cwt", name="cwt")
        c640 = rt.tile([128, 640], BF, tag="c640", name="c640")
        bjm = rt.tile([4, 1280], F32, tag="bjm", name="bjm")
        expjm = rt.tile([4, 1280], BF, tag="expjm", name="expjm")
        expb = rt.tile([128, 40], BF, tag="expb", name="expb")
        rbc32 = rt.tile([BL, 16], F32, tag="rbc32", name="rbc32")
        zc = rt.tile([1, 40], F32, tag="zc", name="zc")
        zs = rt.tile([1, 16], F32, tag="zs", name="zs")
        zrb = rt.tile([1, 16], BF, tag="zrb", name="zrb")
        s_sb = rt.tile([BL, 160], F32, tag="s_sb", name="s_sb")
        sq = rt.tile([BL, 160], F32, tag="sq", name="sq")
        num = rt.tile([BL, 160], F32, tag="num", name="num")
        dn = rt.tile([BL, 160], F32, tag="dn", name="dn")
        v_sb = rt.tile([BL, 160], F32, tag="v_sb", name="v_sb")
        vbig = rt.tile([128, 2560], BF, tag="vbig", name="vbig")
        a_sb = rt.tile([128, 1280], BF, tag="a_sb", name="a_sb")  # [(jm,b),(M,rr,c)]
        braw = rt.tile([4, 1280], BF, tag="braw", name="braw")

        w2c5 = w2c_sb[:].rearrange("p (i jm c o) -> p i jm c o", i=8, jm=4, c=10)

        def emit_s_v(it):
            rhs_s = w2c_sb if it == 0 else cwt
            psS = ps_r.tile([BL, 160], F32, tag="psmix", name="psS", bufs=1)
            for ch in range(32):
                nc.tensor.matmul(psS[:], xT2_sb[:, ch * 32:(ch + 1) * 32],
                                 rhs_s[:, ch * 160:(ch + 1) * 160],
                                 start=(ch == 0), stop=(ch == 31))
            if it == 0:
                nc.scalar.mul(s_sb[:], psS[:], 1.0 / 512.0)
            else:
                zbc = rbc32[:, 0:10].unsqueeze(2).broadcast_to([BL, 10, 16])
                nc.vector.tensor_tensor(s_sb[:].rearrange("p (c o) -> p c o", c=10),
                                        psS[:].rearrange("p (c o) -> p c o", c=10),
                                        zbc, op=ALU.mult)
            nc.vector.tensor_tensor(sq[:], s_sb[:], s_sb[:], op=ALU.mult)
            nc.vector.tensor_tensor(num[:], sq[:], s_sb[:], op=ALU.mult)
            nc.vector.tensor_scalar_add(dn[:], sq[:], 1.0)
            nc.scalar.activation(sq[:], sq[:], ACTF.Sqrt)  # sq <- |s|
            nc.vector.tensor_tensor(dn[:], dn[:], sq[:], op=ALU.mult)
            nc.vector.reciprocal(dn[:], dn[:])
            nc.vector.tensor_tensor(v_sb[:], num[:], dn[:], op=ALU.mult)

        def emit_vrep():
            # vbig[p, (m,rr,c,o)] = v replicated over the 16 (m,rr) positions
            psV = ps_r.tile([128, 160], F32, tag="pss", name="psV", bufs=1)
            nc.tensor.matmul(psV[:], rep4_sb[:], v_sb[:], start=True, stop=True)
            nc.scalar.activation(vbig[:, 0:160], psV[:], ACTF.Copy)
            for ln in (160, 320, 640, 1280):
                nc.vector.tensor_copy(vbig[:, ln:2 * ln], vbig[:, 0:ln])

        def emit_a_chunks(chks):
            # a[p,(M,rr,c)] = sum_o uhat * v ; chunks split DVE / GpSimd
            with nc.allow_low_precision("a-dot bf16 accumulation, tolerance 2e-2"):
                for chk in chks:
                    eng = nc.gpsimd if (chk % 8) >= 5 else nc.vector
                    pool_tag = "tmpg" if (chk % 8) >= 5 else "tmpc"
                    tmpc = rt.tile([128, 2560], BF, tag=pool_tag, name=pool_tag, bufs=2)
                    eng.tensor_tensor(tmpc[:], uhat_sb[:, chk * 2560:(chk + 1) * 2560],
                                      vbig[:], op=ALU.mult)
                    for hw_ in (8, 4, 2):
                        t = tmpc[:].rearrange("p (g o) -> p g o", g=160)
                        eng.tensor_tensor(t[:, :, 0:hw_], t[:, :, 0:hw_],
                                          t[:, :, hw_:2 * hw_], op=ALU.add)
                    t = tmpc[:].rearrange("p (g o) -> p g o", g=160)
                    eng.tensor_tensor(
                        a_sb[:, chk * 160:(chk + 1) * 160].unsqueeze(2),
                        t[:, :, 0:1], t[:, :, 1:2], op=ALU.add)

        def emit_b_update(it):
            # local-batch mean of a (1/BL folded into m4) -> bjm [4,(M,rr,c)]
            psb = ps_r.tile([4, 1280], F32, tag="psmix", name="psb", bufs=1)
            for off, ln in ((0, 512), (512, 512), (1024, 256)):
                nc.tensor.matmul(psb[:, off:off + ln], m4_sb[:],
                                 a_sb[:, off:off + ln], start=True, stop=True)
            if it == 0:
                nc.scalar.activation(bjm[:], psb[:], ACTF.Copy)
            else:
                nc.vector.tensor_tensor(bjm[:], bjm[:], psb[:], op=ALU.add)
            nc.scalar.activation(expjm[:], bjm[:], ACTF.Exp)
            nc.sync.dma_start(bdram[:], expjm[:])
            bsrc = mkap(bdram[:], 0, [[10, 128], [1280, 4], [1, 10]])
            nc.sync.dma_start(expb[:].rearrange("p (jm c) -> p jm c", jm=4), bsrc)

        def emit_cnorm():
            # z_c = sum_r exp(b); rbc32[b,c] = 1/z_c ; cwt = w2c * exp(b)
            psC = ps_r.tile([1, 40], F32, tag="pss", name="psC", bufs=1)
            nc.tensor.matmul(psC[:], ones128_sb[:], expb[:], start=True, stop=True)
            nc.scalar.activation(zc[:], psC[:], ACTF.Copy)
            zcv = zc[:].rearrange("p (jm c) -> p c jm", jm=4)
            nc.vector.tensor_reduce(zs[:, 0:10], zcv, axis=AX.X, op=ALU.add)
            nc.vector.reciprocal(zs[:, 0:10], zs[:, 0:10])
            nc.scalar.activation(zrb[:, 0:10], zs[:, 0:10], ACTF.Copy)
            psB = ps_r.tile([BL, 16], F32, tag="pss", name="psB", bufs=1)
            nc.tensor.matmul(psB[:, 0:10], onesrow_sb[:, 0:BL], zrb[:, 0:10],
                             start=True, stop=True)
            nc.scalar.activation(rbc32[:, 0:10], psB[:, 0:10], ACTF.Copy)
            # c640[p,(jm,c,o)] = exp(b) bcast over o; cwt = w2c * c640 per i
            nc.vector.tensor_copy(
                c640[:].rearrange("p (jc o) -> p jc o", jc=40),
                expb[:].unsqueeze(2).broadcast_to([128, 40, 16]))
            for i in range(8):
                nc.vector.tensor_tensor(cwt[:, i * 640:(i + 1) * 640],
                                        w2c_sb[:, i * 640:(i + 1) * 640],
                                        c640[:], op=ALU.mult)

        # ---- it0: s/v/vrep hoisted (only needs xT2 + w2c) ----
        emit_s_v(0)
        emit_vrep()

        # =================== u_hat: 32 windows, one 128-K matmul each ==========
        with tc.tile_pool(name="w2sp", bufs=1) as w2sp, \
             tc.tile_pool(name="ps_u", bufs=2, space="PSUM") as ps_u:
            w2sall = w2sp.tile([128, 20480], F8, tag="w2sall", name="w2sall")
            for blk in range(8):
                nc.sync.dma_start(w2sall[:, blk * 2560:(blk + 1) * 2560],
                                  P["w2s"][:, blk * 2560:(blk + 1) * 2560])
            for blk in range(8):
                for mm in range(4):
                    m = blk * 4 + mm
                    pst = ps_u.tile([128, 640], F32, tag="ups", name="ups")
                    for q in range(2):
                        nc.tensor.matmul(
                            pst[:, q * 320:(q + 1) * 320],
                            xTbd_sb[:, m * 128:(m + 1) * 128],
                            w2sall[:, m * 640 + q * 320:m * 640 + (q + 1) * 320],
                            start=True, stop=True)
                    nc.scalar.mul(uhat_sb[:, m * 640:(m + 1) * 640], pst[:],
                                  1.0 / 16.0)
                emit_a_chunks([blk])  # it0 a-dot for this block, overlaps PE

        # ---- routing iterations, all-local (validated vs reference) ----
        emit_b_update(0)
        emit_cnorm()
        emit_s_v(1)
        emit_vrep()
        emit_a_chunks(range(8))
        emit_b_update(1)
        emit_cnorm()
        emit_s_v(2)

        # =================== classes / local softmax / argmax -> flat ==========
        nc.vector.tensor_tensor(sq[:], v_sb[:], v_sb[:], op=ALU.mult)
        cl = rt.tile([BL, 10], F32, tag="cl", name="cl")
        nc.vector.tensor_reduce(cl[:], sq[:].rearrange("p (c o) -> p c o", c=10),
                                axis=AX.X, op=ALU.add)
        nc.scalar.activation(cl[:], cl[:], ACTF.Sqrt)
        ecl = rt.tile([BL, 10], F32, tag="ecl", name="ecl")
        nc.scalar.activation(ecl[:], cl[:], ACTF.Exp)
        psZ = ps_r.tile([1, 16], F32, tag="pss", name="psZ", bufs=1)
        nc.tensor.matmul(psZ[:, :10], ones32_sb[:], ecl[:], start=True, stop=True)
        zrow = rt.tile([1, 16], F32, tag="zrow", name="zrow")
        nc.scalar.activation(zrow[:, :10], psZ[:, :10], ACTF.Copy)
        nc.vector.reciprocal(zrow[:, :10], zrow[:, :10])
        zrc = rt.tile([1, 16], BF, tag="zrc", name="zrc")
        nc.scalar.activation(zrc[:, :10], zrow[:, :10], ACTF.Copy)
        psB2 = ps_r.tile([BL, 16], F32, tag="pss", name="psB2", bufs=1)
        nc.tensor.matmul(psB2[:, 0:10], onesrow_sb[:, 0:BL], zrc[:, 0:10],
                         start=True, stop=True)
        zbc32 = rt.tile([BL, 10], F32, tag="zbc32", name="zbc32")
        nc.scalar.activation(zbc32[:], psB2[:, 0:10], ACTF.Copy)
        tpr = rt.tile([BL, 10], F32, tag="tpr", name="tpr")
        nc.vector.tensor_tensor(tpr[:], ecl[:], zbc32[:], op=ALU.mult)
        tmax = rt.tile([BL, 1], F32, tag="tmax", name="tmax")
        nc.vector.tensor_reduce(tmax[:], tpr[:], axis=AX.X, op=ALU.max)
        mask = rt.tile([BL, 10], F32, tag="mask", name="mask")
        nc.vector.tensor_scalar(mask[:], tpr[:], tmax[:], None, op0=ALU.is_equal)
        flat = rt.tile([BL, 160], F32, tag="flat", name="flat")
        mb = mask[:].unsqueeze(2).broadcast_to([BL, 10, 16])
        nc.vector.tensor_tensor(flat[:].rearrange("p (c o) -> p c o", c=10),
                                v_sb[:].rearrange("p (c o) -> p c o", c=10),
                                mb, op=ALU.mult)

        # =================== decoder fc1 / fc2 ===================
        psT = ps_r.tile([128, 32], F32, tag="pss", name="psT", bufs=1)
        nc.tensor.transpose(psT[:], flat[:, 0:128], id32_sb[:])
        fTa = rt.tile([128, 32], BF, tag="fTa", name="fTa")
        nc.scalar.activation(fTa[:], psT[:], ACTF.Copy)
        psT2 = ps_r.tile([32, 32], F32, tag="pss", name="psT2", bufs=1)
        nc.tensor.transpose(psT2[:], flat[:, 128:160], id32_sb[:])
        fTb = rt.tile([32, 32], BF, tag="fTb", name="fTb")
        nc.scalar.activation(fTb[:], psT2[:], ACTF.Copy)
        for fc in range(4):
            ps1 = ps_r.tile([128, 32], F32, tag="pss", name="ps1", bufs=1)
            nc.tensor.matmul(ps1[:], w1ta_sb[:, fc * 128:(fc + 1) * 128], fTa[:],
                             start=True, stop=False)
            nc.tensor.matmul(ps1[:], w1tb_sb[:, fc * 128:(fc + 1) * 128], fTb[:],
                             start=False, stop=True)
            nc.scalar.activation(h1T_sb[:, fc * 32:(fc + 1) * 32], ps1[:],
                                 ACTF.Relu, bias=b1dh_sb[:, fc:fc + 1], scale=1.0)
        # fc2 with h1T as the stationary operand: 8 matmuls of N=512
        w2tv = w2th_sb[:].rearrange("p (kc n) -> p kc n", kc=4)
        with tc.tile_pool(name="ps_d", bufs=1, space="PSUM") as ps_d:
            psH2 = [ps_d.tile([32, 512], F32, tag=f"psh2{nh}", name=f"psh2{nh}", bufs=1)
                    for nh in range(2)]
            for kc in range(4):
                for nh in range(2):
                    nc.tensor.matmul(psH2[nh][:], h1T_sb[:, kc * 32:(kc + 1) * 32],
                                     w2tv[:, kc, nh * 512:(nh + 1) * 512],
                                     start=(kc == 0), stop=(kc == 3))
            h2braw = rt.tile([32, 1024], F32, tag="h2braw", name="h2braw")
            for nh in range(2):
                nc.scalar.activation(h2braw[:, nh * 512:(nh + 1) * 512], psH2[nh][:],
                                     ACTF.Copy)
            # transpose to feature-major, relu+bias+scale into fp8
            for gc in range(8):
                psT3 = ps_d.tile([128, 32], F32, tag="psT3", name="psT3", bufs=2)
                nc.tensor.transpose(psT3[:], h2braw[:, gc * 128:(gc + 1) * 128], id32_sb[:])
                nc.scalar.activation(h2T_sb[:, gc * 32:(gc + 1) * 32], psT3[:],
                                     ACTF.Relu, bias=b2dh_sb[:, gc:gc + 1], scale=SH)
        h2dst = mkap(h2loc[:], 0, [[BL, 128], [128 * BL, 8], [1, BL]])
        nc.sync.dma_start(h2dst, h2T_sb[:])
        nc.gpsimd.collective_compute(
            "AllGather", ALU.bypass, replica_groups=rg,
            ins=[h2loc[:].opt()], outs=[h2all[:].opt()])

    psr_cm.__exit__(None, None, None)
    rt_cm.__exit__(None, None, None)

    # =================== final big layer (fp8 DoubleRow, tensor-parallel) ======
    with tc.tile_pool(name="fin", bufs=1) as fin, \
         tc.tile_pool(name="ps_o", bufs=4, space="PSUM") as ps_o, \
         tc.tile_pool(name="osb", bufs=4) as osbp:
        ldq = [fin.tile([128, 512], F8, tag=f"ldq{q}", name=f"ldq{q}") for q in range(4)]
        for q in range(4):
            for i in range(2):
                src = mkap(h2all[:], (q * 2 + i) * 128 * BL,
                           [[BL, 128], [8 * 128 * BL, NCORES], [1, BL]])
                nc.sync.dma_start(
                    ldq[q][:, i * 256:(i + 1) * 256].rearrange(
                        "p (cc b) -> p cc b", cc=8), src)
        for w in range(NW):
            if w + NPRE < NW:
                # software-pipelined stream: issue the load NPRE windows ahead
                nt = w3p.tile([128, 4096], F8, tag="w3t", name="w3t")
                w3_load(w + NPRE, nt)
                w3tiles.append(nt)
            wt3 = w3tiles[w]
            w3v = wt3[:].rearrange("p (q i n) -> p q i n", q=4, i=2)
            if use_b3:
                b3w = fin.tile([1, 512], BF, tag="b3w", name="b3w", bufs=2)
                nc.sync.dma_start(b3w[:], P["b3s"][:, w * 512:(w + 1) * 512])
            for bh in range(2):
                pso = ps_o.tile([128, 512], F32, tag="pso", name="pso")
                for q in range(4):
                    lhs = ldq[q][:].rearrange("p (i b) -> p i b", i=2)[
                        :, :, bh * 128:(bh + 1) * 128]
                    nc.tensor.matmul(pso[:], lhs, w3v[:, q], start=(q == 0),
                                     stop=(q == 3 and not use_b3), perf_mode=DR)
                if use_b3:
                    nc.tensor.matmul(pso[:], onesrow_sb[:], b3w[:],
                                     start=False, stop=True, skip_group_check=True)
                ot = osbp.tile([128, 512], BF, tag="ot", name="ot")
                nc.scalar.activation(ot[:], pso[:], ACTF.Sigmoid, scale=1.0 / (S3 * SH))
                nc.scalar.dma_start(out_ext[bh * 128:(bh + 1) * 128,
                                            w * 512:(w + 1) * 512], ot[:])

    w3cm.__exit__(None, None, None)
    dram_cm.__exit__(None, None, None)
    es.__exit__(None, None, None)


_NC_CACHE = {}


def _host_prep(inputs, use_b3):
    data = np.asarray(inputs["data"], np.float32)
    conv1_w = np.asarray(inputs["conv1_w"], np.float32)
    conv1_b = np.asarray(inputs["conv1_b"], np.float32)
    prim_w = np.asarray(inputs["prim_w"], np.float32)
    prim_b = np.asarray(inputs["prim_b"], np.float32)
    W_digit = np.asarray(inputs["W_digit"], np.float32)
    dec_w1 = np.asarray(inputs["dec_w1"], np.float32)
    dec_b1 = np.asarray(inputs["dec_b1"], np.float32)
    dec_w2 = np.asarray(inputs["dec_w2"], np.float32)
    dec_b2 = np.asarray(inputs["dec_b2"], np.float32)
    dec_w3 = np.asarray(inputs["dec_w3"], np.float32)
    dec_b3 = np.asarray(inputs["dec_b3"], np.float32)

    w1c = np.ascontiguousarray(conv1_w[:, 0].transpose(1, 2, 0).reshape(81, 256)
                               ).astype(BF16)
    wp2 = np.ascontiguousarray(prim_w.transpose(2, 3, 1, 0).reshape(20736, 256))
    wp2q = np.clip(wp2 * S1, -240, 240).astype(FP8)
    # route mapping: r = jm*128 + m*4 + rr
    W2 = np.ascontiguousarray(W_digit.transpose(0, 3, 1, 2).reshape(512, 8, 160))
    w2s = np.zeros((128, 32, 4, 160), np.float32)
    marr = np.arange(32)
    for jm in range(4):
        for rr in range(4):
            for i in range(8):
                w2s[32 * jm + rr * 8 + i, :, rr, :] = W2[jm * 128 + marr * 4 + rr, i, :]
    w2s = np.clip(w2s.reshape(128, 20480) * 16.0, -240, 240).astype(FP8)
    # w2c[p, (i, jm, c, o)] = W_digit[jm*128+p, c, o, i]
    w2c = np.ascontiguousarray(
        W_digit.reshape(4, 128, 10, 16, 8).transpose(1, 4, 0, 2, 3).reshape(128, 5120)
    ).astype(BF16)
    # 1/BL folded in: the m4 matmul then yields the local-batch mean directly
    m4 = (np.repeat(np.eye(4, dtype=np.float32), 32, axis=0) / BL).astype(BF16)
    rep4 = np.tile(np.eye(32, dtype=np.float32), (1, 4))
    w1t = np.ascontiguousarray(dec_w1.T).astype(BF16)
    w2t = np.ascontiguousarray(dec_w2.T).astype(BF16)
    w3t = np.ascontiguousarray(dec_w3.T)

    common = dict(
        w1c=w1c, b1c2=np.ascontiguousarray(conv1_b.reshape(2, 128).T), wp2q=wp2q,
        bp22=np.ascontiguousarray(prim_b.reshape(2, 128).T),
        w2s=w2s, w2c=w2c, m4=m4, rep4=rep4,
        ones32=np.ones((32, 1), np.float32),
        ones128=np.ones((128, 1), np.float32).astype(BF16),
        onesrow=np.ones((1, 128), np.float32).astype(BF16),
        id32=np.eye(32, dtype=np.float32),
        w1t=w1t, b1dh=np.ascontiguousarray(dec_b1.reshape(4, 128).T),
        w2t=w2t, b2dh=np.ascontiguousarray(dec_b2.reshape(8, 128).T) * SH,
    )
    in_maps = []
    for c in range(NCORES):
        m = dict(common)
        sw = np.lib.stride_tricks.sliding_window_view(
            data[c * BL:(c + 1) * BL, 0], (9, 9), axis=(1, 2))
        m["pat1h"] = np.ascontiguousarray(
            sw.transpose(3, 4, 1, 2, 0).reshape(81, 4608)).astype(BF16)
        m["w3q"] = np.clip(w3t[:, c * RECL:(c + 1) * RECL] * S3, -240, 240).astype(FP8)
        if use_b3:
            m["b3s"] = (dec_b3[c * RECL:(c + 1) * RECL] * (S3 * SH)).reshape(1, RECL).astype(BF16)
        in_maps.append(m)
    return in_maps


def kernel(**inputs):
    use_b3 = bool(np.any(np.asarray(inputs["dec_b3"], np.float32)))
    key = ("nc", use_b3)
    if key not in _NC_CACHE:
        _NC_CACHE[key] = build_program(use_b3)
    nc = _NC_CACHE[key]
    in_maps = _host_prep(inputs, use_b3)
    res = run_bass_kernel_spmd(nc, in_maps, list(range(NCORES)))
    outs = [res.results[c]["out"] for c in range(NCORES)]
    rec = np.concatenate(outs, axis=1).astype(np.float32)
    return rec.reshape(B, 256, 20, 20)
